# revision 7
# baseline (speedup 1.0000x reference)
"""
Trainium2 Bass kernel for the Decoder_RNN_Simple problem.

Math (per flat-batch element b, reference semantics):
  hidden0 = tanh(W_z0 @ z0 + b_z0)                       # [256]
  cur0 = 0
  for t in 0..199:
    x = [cur, tps[t]]                                    # [65]
    gx = W_ih @ x + b_ih ; gh = W_hh @ hidden + b_hh     # [768]
    r = sig(gx_r + gh_r); z = sig(gx_z + gh_z)
    n = tanh(gx_n + r * gh_n)
    h' = (1-z)*n + z*h ; pred = W_out @ h' + b_out       # [64]

Mapping (data-parallel over the flat batch of 8192 across 8 cores,
1024 rows per core; transposed [gates, batch] on-chip layout so the
recurrence needs no transposes):

  - cur_t = pred_{t-1} = W_out @ h_t + b_out for t>=1, so the r/z gate
    pre-activations fold into a single matmul with
    W_eff = W_hh + W_ih[:, :64] @ W_out applied to h (exact algebra).
    The n gate keeps xn (from the pred tile, K=64) and hn (from raw
    W_hh) separate since n = tanh(xn + r*hn).
  - Recurrent state h and the r/z/n matmul weights are bf16 (validated
    ~4e-3 rel err vs fp32 host sim); PSUM accumulation stays fp32.
  - Per-gate biases (b_ih + b_hh + W_ih[:,64]*tps[t] (+ W_ih[:,:64] @
    b_out for t>=1)) are precomputed host-side as [gate, 200] tables and
    applied through the ACT bias operand. b_out itself is added to the
    output host-side after the gather.
  - The n-gate sum t2 = (hn + b_hhn)*r + xn is built by an in-place
    scalar_tensor_tensor on the hn PSUM bank followed by the xn matmul
    accumulating into the same bank (start=False), so no extra DVE pass.
  - One step's work per gru-chunk c (128 rows) runs at full batch width
    1024; matmuls split N=512 to respect the PSUM bank size.
"""

import sys

_TRN = "/opt/trn_rl_repo"
if _TRN not in sys.path:
    sys.path.insert(0, _TRN)

import numpy as np
import ml_dtypes

import concourse.bass as bass
import concourse.mybir as mybir
import concourse.tile as tile
from concourse.vector_clock import ScopedClock
from concourse.bass_utils import run_bass_kernel_spmd

N_CORES = 8
LATENT = 128
OUT_DIM = 64
N_GRU = 256
N_TP = 200
B_FULL = 64 * 128
B_LOC = B_FULL // N_CORES  # 1024
HALF = 512
F32 = mybir.dt.float32
F32R = mybir.dt.float32r
BF16 = mybir.dt.bfloat16

AF = mybir.ActivationFunctionType
ALU = mybir.AluOpType


# walrus rejects sem waits carried on the kernel-tail Drain instruction
# ("Too many sync wait commands"); move them onto NOPs, one wait each.
def _patched_drain_and_barrier(self, tick_clock, wait_clock):
    carrier = self.nc.sync.nop()
    wait_clock.add_sem_waits(carrier.ins, ScopedClock({None: tick_clock.global_clock}))
    si = carrier.ins.sync_info
    waits = list(si.on_wait) if si is not None else []
    if len(waits) > 1:
        si.on_wait = waits[:1]
        rest = waits[1:]
        while rest:
            extra = self.nc.sync.nop()
            extra.ins.sync_info = mybir.SyncInfo(on_wait=rest[:1], on_update=[])
            rest = rest[1:]
    self.nc.sync.drain()
    self.nc.all_engine_barrier()
    popped = self.nc._tile_sem_poison_stack.pop()
    assert popped is self._sem_poison
    self.nc.clear_and_free_semaphores(list(self.sems.allocated().values()))
    self.nc.all_engine_barrier()


tile.TileContext._drain_and_barrier = _patched_drain_and_barrier


def _split_waits(nc, maxw=1):
    """This walrus rejects instructions carrying more than a couple of sem
    waits; move the excess onto same-engine NOPs inserted just before."""
    k = 0
    for f in nc.m.functions:
        for bb in f.blocks:
            insts = bb.instructions
            out = []
            changed = False
            for inst in insts:
                si = inst.sync_info
                waits = list(si.on_wait) if si is not None else []
                if len(waits) > maxw:
                    si.on_wait = waits[-maxw:]
                    excess = waits[:-maxw]
                    while excess:
                        chunk, excess = excess[:maxw], excess[maxw:]
                        nop = mybir.InstNoOp(name=f"waitsplit_{k}", ins=[], outs=[])
                        k += 1
                        nop.engine = inst.engine
                        nop.sync_info = mybir.SyncInfo(on_wait=chunk, on_update=[])
                        out.append(nop)
                    changed = True
                out.append(inst)
            if changed:
                bb.instructions = out
    return k


def _build_module(repeat=1, split_waits=True):
    nc = bass.Bass("TRN2", target_bir_lowering=False, debug=False, num_devices=N_CORES)

    def inp(name, shape, dt=F32):
        return nc.dram_tensor(name, shape, dt, kind="ExternalInput").ap()

    d = {
        "z0t": inp("z0t", [LATENT, B_LOC]),
        "wz0t": inp("wz0t", [LATENT, N_GRU]),
        "whht1": inp("whht1", [N_GRU, 3 * N_GRU], BF16),  # eff r,z; raw n
        "whht0": inp("whht0", [N_GRU, 2 * N_GRU], BF16),  # raw r,z (step 0)
        "wxnt": inp("wxnt", [OUT_DIM, N_GRU]),
        "woutt": inp("woutt", [N_GRU, OUT_DIM], BF16),
        "brz": inp("brz", [2 * N_GRU, N_TP]),
        "bxn": inp("bxn", [N_GRU, N_TP]),
        "bhhn": inp("bhhn", [N_GRU, 1]),
        "bz0": inp("bz0", [N_GRU, 1]),
    }
    out = nc.dram_tensor("out", [N_TP, OUT_DIM, B_LOC], F32, kind="ExternalOutput").ap()

    with tile.TileContext(nc) as tc:
        for _ in range(repeat):
            _emit(nc, tc, d, out)
    if split_waits:
        n = _split_waits(nc, maxw=1)
        print(f"[kernel] split {n} excess sem-waits onto NOPs", flush=True)
    return nc


def _emit(nc, tc, d, out):
    B = B_LOC
    with (
        tc.tile_pool(name="const", bufs=1) as cp,
        tc.tile_pool(name="work", bufs=2) as wp,
        tc.tile_pool(name="psum", bufs=3, space="PSUM") as pp,
        tc.tile_pool(name="ppred", bufs=1, space="PSUM") as ppr,
    ):
        def const_tile(name, shape, dt=F32):
            t = cp.tile(shape, dt, tag=name)
            dma = nc.gpsimd if dt is F32R else nc.sync
            dma.dma_start(t[:], d[name][:])
            return t

        def const_rows(name, shape, r0, tag, dt=F32):
            t = cp.tile(shape, dt, tag=tag)
            dma = nc.gpsimd if dt is F32R else nc.sync
            dma.dma_start(t[:], d[name][r0 : r0 + shape[0], :])
            return t

        wz0 = const_tile("wz0t", [LATENT, N_GRU], F32R)
        whh1 = [const_rows("whht1", [128, 3 * N_GRU], 128 * k, f"whh1_{k}", BF16) for k in range(2)]
        whh0 = [const_rows("whht0", [128, 2 * N_GRU], 128 * k, f"whh0_{k}", BF16) for k in range(2)]
        wxn = const_tile("wxnt", [OUT_DIM, N_GRU], F32R)
        wout = [const_rows("woutt", [128, OUT_DIM], 128 * k, f"wout_{k}", BF16) for k in range(2)]
        brz = [const_rows("brz", [128, N_TP], 128 * g, f"brz_{g}") for g in range(4)]
        bxn = [const_rows("bxn", [128, N_TP], 128 * c, f"bxn_{c}") for c in range(2)]
        bhhn = [const_rows("bhhn", [128, 1], 128 * c, f"bhhn_{c}") for c in range(2)]
        bz0 = [const_rows("bz0", [128, 1], 128 * c, f"bz0_{c}") for c in range(2)]

        # persistent pred tile in f32r for the xn matmul + DMA out
        pred_sb = cp.tile([OUT_DIM, B], F32R, tag="pred_sb")

        # ---- initial hidden: h = tanh(Wz0 @ z0T + b_z0), [256, B] as 2 chunks
        z0sb = wp.tile([LATENT, B], F32R, tag="z0")
        nc.gpsimd.dma_start(z0sb[:], d["z0t"][:])
        h = [None, None]
        for c in range(2):
            p = pp.tile([128, B], F32, tag="ps")
            for hf in range(2):
                bs = slice(hf * HALF, (hf + 1) * HALF)
                nc.tensor.matmul(p[:, bs], wz0[:, c * 128 : (c + 1) * 128], z0sb[:, bs],
                                 start=True, stop=True)
            hc = wp.tile([128, B], BF16, tag=f"h{c}")
            nc.scalar.activation(hc[:], p[:], AF.Tanh, bias=bz0[c][:, 0:1])
            h[c] = hc

        def rz_mms(g, wk, dst):
            col = slice(g * 128, (g + 1) * 128)
            for hf in range(2):
                bs = slice(hf * HALF, (hf + 1) * HALF)
                nc.tensor.matmul(dst[:, bs], wk[0][:, col], h[0][:, bs],
                                 start=True, stop=False)
                nc.tensor.matmul(dst[:, bs], wk[1][:, col], h[1][:, bs],
                                 start=False, stop=True)

        def hn_mms(c, dst, last):
            col = slice(512 + c * 128, 512 + (c + 1) * 128)
            for hf in range(2):
                bs = slice(hf * HALF, (hf + 1) * HALF)
                nc.tensor.matmul(dst[:, bs], whh1[0][:, col], h[0][:, bs],
                                 start=True, stop=False)
                nc.tensor.matmul(dst[:, bs], whh1[1][:, col], h[1][:, bs],
                                 start=False, stop=last)

        for t in range(N_TP):
            first = t == 0
            wk = whh0 if first else whh1

            # r gates (needed first by stt), then z; hn interleaved on PE
            prz = [None] * 4
            sig = [None] * 4
            pn = [None, None]
            for g in (0, 1):
                pt = pp.tile([128, B], F32, tag="ps")
                prz[g] = pt
                rz_mms(g, wk, prz[g])
                sg = wp.tile([128, B], BF16, tag=f"sig{g}")
                nc.scalar.activation(sg[:], prz[g][:], AF.Sigmoid,
                                     bias=brz[g][:, t : t + 1])
                sig[g] = sg
            pn0 = pp.tile([128, B], F32, tag="ps")
            pn[0] = pn0
            hn_mms(0, pn[0], last=True)
            for g in (2, 3):
                pt = pp.tile([128, B], F32, tag="ps")
                prz[g] = pt
                rz_mms(g, wk, prz[g])
                sg = wp.tile([128, B], BF16, tag=f"sig{g}")
                nc.scalar.activation(sg[:], prz[g][:], AF.Sigmoid,
                                     bias=brz[g][:, t : t + 1])
                sig[g] = sg
            pn1 = pp.tile([128, B], F32, tag="ps")
            pn[1] = pn1
            hn_mms(1, pn[1], last=True)

            # n gate: t2 = (hn + b_hhn)*r (+ xn via PE accumulate), tanh+bias
            n_ = [None, None]
            for c in range(2):
                nc.vector.scalar_tensor_tensor(
                    pn[c][:], pn[c][:], bhhn[c][:, 0:1], sig[c][:],
                    ALU.add, ALU.mult,
                )
                if not first:
                    xcol = slice(c * 128, (c + 1) * 128)
                    for hf in range(2):
                        bs = slice(hf * HALF, (hf + 1) * HALF)
                        nc.tensor.matmul(pn[c][:, bs], wxn[:, xcol], pred_sb[:, bs],
                                         start=False, stop=True,
                                         skip_group_check=True)
                nt = wp.tile([128, B], BF16, tag=f"n{c}")
                nc.scalar.activation(nt[:], pn[c][:], AF.Tanh,
                                     bias=bxn[c][:, t : t + 1])
                n_[c] = nt

            # h' = n + z*(h - n); sub on GPSIMD, rest on DVE (bf16 2x mode)
            h_new = [None, None]
            for c in range(2):
                dt_ = wp.tile([128, B], BF16, tag=f"d{c}")
                nc.gpsimd.tensor_sub(dt_[:], h[c][:], n_[c][:])
                e = wp.tile([128, B], BF16, tag=f"e{c}")
                nc.vector.tensor_mul(e[:], sig[2 + c][:], dt_[:])
                hc = wp.tile([128, B], BF16, tag=f"h{c}")
                nc.vector.tensor_add(hc[:], e[:], n_[c][:])
                h_new[c] = hc
            h = h_new

            # pred = W_out @ h' (b_out added host-side) -> SBUF + DRAM out
            ppt = ppr.tile([OUT_DIM, B], F32, tag="pp")
            for hf in range(2):
                bs = slice(hf * HALF, (hf + 1) * HALF)
                nc.tensor.matmul(ppt[:, bs], wout[0][:, :], h[0][:, bs],
                                 start=True, stop=False)
                nc.tensor.matmul(ppt[:, bs], wout[1][:, :], h[1][:, bs],
                                 start=False, stop=True)
            nc.vector.tensor_scalar_add(pred_sb[:], ppt[:], 0.0)
            nc.sync.dma_start(out[t][:, :], pred_sb[:].bitcast(F32))


_CACHE = {}


def _prep_host(z0, tps_to_pred, W_z0, b_z0, W_ih, b_ih, W_hh, b_hh, W_out, b_out):
    f = np.float32
    bf = ml_dtypes.bfloat16
    z0 = np.asarray(z0, f)
    tps = np.asarray(tps_to_pred, f)
    W_z0, b_z0 = np.asarray(W_z0, f), np.asarray(b_z0, f)
    W_ih, b_ih = np.asarray(W_ih, f), np.asarray(b_ih, f)
    W_hh, b_hh = np.asarray(W_hh, f), np.asarray(b_hh, f)
    W_out, b_out = np.asarray(W_out, f), np.asarray(b_out, f)

    Wihp = W_ih[:, :OUT_DIM]  # [768, 64]
    wt = W_ih[:, OUT_DIM]  # [768]
    G2 = 2 * N_GRU
    Weff_rz = W_hh[:G2] + Wihp[:G2] @ W_out  # [512, 256]
    whht1 = np.ascontiguousarray(
        np.concatenate([Weff_rz, W_hh[G2:]], axis=0).T
    ).astype(bf)  # [256, 768]
    whht0 = np.ascontiguousarray(W_hh[:G2].T).astype(bf)  # [256, 512]
    wxnt = np.ascontiguousarray(Wihp[G2:].T)  # [64, 256]
    woutt = np.ascontiguousarray(W_out.T).astype(bf)  # [256, 64]

    cb = Wihp @ b_out  # [768]
    bias_all = b_ih[:, None] + wt[:, None] * tps[None, :]  # [768, 200]
    brz = bias_all[:G2] + b_hh[:G2, None]
    brz[:, 1:] += cb[:G2, None]
    bxn = bias_all[G2:].copy()
    bxn[:, 1:] += cb[G2:, None]

    shared = {
        "wz0t": np.ascontiguousarray(W_z0.T),
        "whht1": whht1,
        "whht0": whht0,
        "wxnt": wxnt,
        "woutt": woutt,
        "brz": np.ascontiguousarray(brz, f),
        "bxn": np.ascontiguousarray(bxn, f),
        "bhhn": np.ascontiguousarray(b_hh[G2:].reshape(N_GRU, 1)),
        "bz0": np.ascontiguousarray(b_z0.reshape(N_GRU, 1)),
    }
    z0f = z0.reshape(B_FULL, LATENT)
    in_maps = []
    for i in range(N_CORES):
        m = dict(shared)
        m["z0t"] = np.ascontiguousarray(z0f[i * B_LOC : (i + 1) * B_LOC].T)
        in_maps.append(m)
    return in_maps, b_out


def _run(in_maps, repeat=1, **spmd_kwargs):
    key = f"nc{repeat}"
    if key not in _CACHE:
        _CACHE[key] = _build_module(repeat)
    return run_bass_kernel_spmd(_CACHE[key], in_maps, list(range(N_CORES)), **spmd_kwargs)


def _gather(res, b_out):
    outp = np.empty((B_FULL, N_TP, OUT_DIM), np.float32)
    for i in range(N_CORES):
        o = res.results[i]["out"]  # [200, 64, 1024]
        outp[i * B_LOC : (i + 1) * B_LOC] = np.asarray(o).transpose(2, 0, 1)
    outp += b_out[None, None, :]
    return outp.reshape(64, 128, N_TP, OUT_DIM)


def kernel(**inputs):
    in_maps, b_out = _prep_host(**inputs)
    res = _run(in_maps)
    return _gather(res, b_out)


def kernel_profiled(**inputs):
    """Like kernel(), but requests an NTFF trace; returns (output, results)."""
    in_maps, b_out = _prep_host(**inputs)
    res = _run(in_maps, trace=True)
    return _gather(res, b_out), res
